# revision 30
# baseline (speedup 1.0000x reference)
"""Trainium2 Bass kernel for a causal attention block (B=2, T=2048, E=2048,
16 heads, head_dim=128, interleaved RoPE).

Sharding: data-parallel over batch (2) x tensor-parallel over heads (4 per
core) = 8 NeuronCores. Each core computes QKV projection for its 4 heads,
RoPE, causal SDPA, and a partial output projection (row-sharded W_out); the
host sums the 4 TP partials per batch element.

Single fully-pipelined instruction stream (fp16 matmul inputs, fp32 PSUM):
  slot 0:  QKV for t4=0 with e-major accumulation (PE starts as soon as the
           first x/W chunk DMA lands; 8 QK PSUM banks live).
  slot t (1..3): QKV chains for t4=t are woven as PE filler between the
           SDPA score/AV groups of tq=t-1, so ACT (exp) latency never stalls
           the PE. Out-projection chunks for tq=t-2 join the filler pool.
  slot 4:  SDPA tq=3 with proj chunks 8..11 as filler, then proj 12..15.

SDPA is causally trimmed at 128-granularity: diagonal-band blocks compute
only the valid trailing query range (53.1% of full score work instead of
62.5% with 512-wide tiles). Softmax denominators: fp16 tile-sum tree on DVE,
partition_all_reduce on gpsimd (no PE ones-matmuls, no separate broadcast),
fast reciprocal + normalize on DVE.
"""

import sys

sys.path.insert(0, "/opt/trn_rl_repo")

import numpy as np

import concourse.bacc as bacc
import concourse.mybir as mybir
from concourse.alu_op_type import AluOpType
from concourse import bass_isa
from concourse import tile
from concourse import bass_utils

B, T, E = 2, 2048, 2048
N_HEAD = 16
D = E // N_HEAD            # 128
THETA = 10000.0
N_CORES = 8
TP = 4                     # tensor-parallel degree (heads)
HPC = N_HEAD // TP         # heads per core = 4
FL = HPC * D               # local head width = 512
EC = E // 128              # 16 contraction chunks
TQ = 512                   # query tile (free dim)
NTQ = T // TQ              # 4
NTK = T // 128             # 16

F32 = mybir.dt.float32
F16 = mybir.dt.float16
EXP = mybir.ActivationFunctionType.Exp
SCALE = 1.0 / np.sqrt(D)

_compiled = None
_last_in_maps = None


def _build():
    nc = bacc.Bacc("TRN2", target_bir_lowering=False)

    xT = nc.dram_tensor("xT", (E, T), F16, kind="ExternalInput")
    wqk = nc.dram_tensor("wqk", (EC, 128, 2 * FL), F16, kind="ExternalInput")
    wv = nc.dram_tensor("wv", (EC, 128, FL), F16, kind="ExternalInput")
    wout = nc.dram_tensor("wout", (HPC, 128, E), F16, kind="ExternalInput")
    csx = nc.dram_tensor("csx", (128, T), F16, kind="ExternalInput")
    csx2 = nc.dram_tensor("csx2", (128, T), F16, kind="ExternalInput")
    maskx = nc.dram_tensor("maskx", (128, 128), F16, kind="ExternalInput")
    onesx = nc.dram_tensor("onesx", (128, 1), F16, kind="ExternalInput")
    out = nc.dram_tensor("out", (T, E), F16, kind="ExternalOutput")

    with tile.TileContext(nc) as tc, nc.allow_low_precision(
        reason="fp16 matmul inputs / fp16 softmax stats are intentional"
    ):
        with tc.tile_pool(name="const", bufs=1) as const, \
             tc.tile_pool(name="qkt_p", bufs=1) as qkt_p, \
             tc.tile_pool(name="v_p", bufs=1) as v_p, \
             tc.tile_pool(name="yt_p", bufs=1) as yt_p, \
             tc.tile_pool(name="w_p", bufs=1) as w_p, \
             tc.tile_pool(name="xt_p", bufs=2) as xt_p, \
             tc.tile_pool(name="rope_p", bufs=2) as rope_p, \
             tc.tile_pool(name="es_p", bufs=2) as es_p, \
             tc.tile_pool(name="dn_p", bufs=2) as dn_p, \
             tc.tile_pool(name="o_ev", bufs=4) as o_ev:

            cs_sb = const.tile([128, T], F16, tag="cs")    # [cos; sin]
            csd_sb = const.tile([128, T], F16, tag="csd")  # [sin; cos]
            mask_sb = const.tile([128, 128], F16, tag="mask")
            ones_sb = const.tile([128, 1], F16, tag="ones")

            qkt_sb = [[qkt_p.tile([128, TQ], F16, tag=f"qkt{f}_{t}",
                                  name=f"qkt_sb{f}_{t}")
                       for t in range(NTQ)] for f in range(2 * HPC)]
            v_sb = [v_p.tile([128, FL], F16, tag=f"v{t}", name=f"v_sb{t}")
                    for t in range(NTK)]
            yt_sb = [[yt_p.tile([128, TQ], F16, tag=f"yt{h}_{t}",
                                name=f"yt_sb{h}_{t}")
                      for t in range(NTQ)] for h in range(HPC)]
            wqk_sb = [w_p.tile([128, 2 * FL], F16, tag=f"wqk{e}",
                               name=f"wqk_sb{e}") for e in range(EC)]
            wv_sb = [w_p.tile([128, FL], F16, tag=f"wv{e}", name=f"wv_sb{e}")
                     for e in range(EC)]
            wo_sb = [w_p.tile([128, E], F16, tag=f"wo{h}", name=f"wo_sb{h}")
                     for h in range(HPC)]

            xt_sb = {}  # (t4, e) -> tile

            def dma_x_slab(t4):
                for e in range(EC):
                    xt = xt_p.tile([128, TQ], F16, tag=f"xt{e}")
                    nc.sync.dma_start(
                        xt[:], xT[e * 128:(e + 1) * 128,
                                  t4 * TQ:(t4 + 1) * TQ])
                    xt_sb[(t4, e)] = xt

            def rope_evict(ps, f8, t4):
                # PSUM [128,TQ] f32 -> fp16 copy -> RoPE rotation on DVE
                # (all-fp16 SBUF operands; inputs pairwise share a base
                # partition as the ALU engines require)
                ts4 = slice(t4 * TQ, (t4 + 1) * TQ)
                qk16 = rope_p.tile([128, TQ], F16, tag="qk16")
                nc.scalar.copy(qk16[:], ps[:])
                dst = qkt_sb[f8][t4]
                t_a = rope_p.tile([64, TQ], F16, tag="ta", bufs=1)
                t_b = rope_p.tile([64, TQ], F16, tag="tb", bufs=1)
                nc.vector.tensor_tensor(t_a[:], qk16[0:64, :],
                                        cs_sb[0:64, ts4], op=AluOpType.mult)
                nc.vector.tensor_tensor(t_b[:], qk16[64:128, :],
                                        cs_sb[64:128, ts4], op=AluOpType.mult)
                nc.vector.tensor_tensor(dst[0:64, :], t_a[:], t_b[:],
                                        op=AluOpType.subtract)
                t_c = rope_p.tile([64, TQ], F16, tag="tc", bufs=1)
                t_d = rope_p.tile([64, TQ], F16, tag="td", bufs=1)
                nc.vector.tensor_tensor(t_c[:], qk16[0:64, :],
                                        csd_sb[0:64, ts4], op=AluOpType.mult)
                nc.vector.tensor_tensor(t_d[:], qk16[64:128, :],
                                        csd_sb[64:128, ts4], op=AluOpType.mult)
                nc.vector.tensor_tensor(dst[64:128, :], t_c[:], t_d[:],
                                        op=AluOpType.add)

            # ---------------- DMA preamble (priority order) ----------------
            # dma_start issue costs ~0.6us on the sync queue (8 outstanding),
            # so keep the count low and issue the first compute's chunks
            # before anything else.
            for e in range(8):
                xt = xt_p.tile([128, TQ], F16, tag=f"xt{e}")
                nc.sync.dma_start(xt[:], xT[e * 128:(e + 1) * 128, 0:TQ])
                xt_sb[(0, e)] = xt
                nc.sync.dma_start(wqk_sb[e][:], wqk[e])
            nc.sync.dma_start(cs_sb[:], csx[:])
            nc.sync.dma_start(csd_sb[:], csx2[:])
            nc.sync.dma_start(mask_sb[:], maskx[:])
            nc.sync.dma_start(ones_sb[:], onesx[:])
            for e in range(8, EC):
                xt = xt_p.tile([128, TQ], F16, tag=f"xt{e}")
                nc.sync.dma_start(xt[:], xT[e * 128:(e + 1) * 128, 0:TQ])
                xt_sb[(0, e)] = xt
                nc.sync.dma_start(wqk_sb[e][:], wqk[e])
            for e in range(EC):
                nc.sync.dma_start(wv_sb[e][:], wv[e])

            # ---------------- slot 0: QKV for t4=0, e-major ----------------
            # Two e-major half-passes (f8 0..3 then 4..7) so the RoPE
            # evictions of each half drain on ACT/DVE under the next pass's
            # matmuls instead of bunching at a single loop end.
            with tc.tile_pool(name="ps0", bufs=1, space="PSUM") as ps0:
                qkps = [ps0.tile([128, TQ], F32, tag=f"qk{f8}",
                                 name=f"qkps{f8}") for f8 in range(2 * HPC)]
                for e in range(EC):
                    for f8 in range(HPC):
                        nc.tensor.matmul(
                            qkps[f8][:],
                            wqk_sb[e][:, f8 * 128:(f8 + 1) * 128],
                            xt_sb[(0, e)][:],
                            start=(e == 0), stop=(e == EC - 1),
                            skip_group_check=True,
                        )
                # prefetch next x slab + wout while evictions/V run
                dma_x_slab(1)
                for h in range(HPC):
                    nc.sync.dma_start(wo_sb[h][:], wout[h])
                for f8 in range(HPC):
                    rope_evict(qkps[f8], f8, 0)
                for e in range(EC):
                    for f8 in range(HPC, 2 * HPC):
                        nc.tensor.matmul(
                            qkps[f8][:],
                            wqk_sb[e][:, f8 * 128:(f8 + 1) * 128],
                            xt_sb[(0, e)][:],
                            start=(e == 0), stop=(e == EC - 1),
                            skip_group_check=True,
                        )
                for f8 in range(HPC, 2 * HPC):
                    rope_evict(qkps[f8], f8, 0)
                for i in range(4):
                    ps = ps0.tile([128, FL], F32, tag=f"qk{i}",
                                  name=f"vps{i}")
                    for e in range(EC):
                        nc.tensor.matmul(
                            ps[:], xt_sb[(0, e)][:, i * 128:(i + 1) * 128],
                            wv_sb[e][:],
                            start=(e == 0), stop=(e == EC - 1),
                            skip_group_check=True,
                        )
                    nc.scalar.copy(v_sb[i][:], ps[:])
                dma_x_slab(2)

            # ---------------- slots 1..4: pipelined SDPA ----------------
            # ps_qkv (2 banks) lives only through slot 2; its banks are then
            # recycled into a 2-deep yps ring for slots 3-4 so a head's first
            # AV matmul never waits the previous head's normalize (DVE).
            with tc.tile_pool(name="ps_sc", bufs=3, space="PSUM") as ps_sc, \
                 tc.tile_pool(name="ps_y", bufs=1, space="PSUM") as ps_y, \
                 tc.tile_pool(name="ps_o", bufs=2, space="PSUM") as ps_o:
                ps_qkv_cm = tc.tile_pool(name="ps_qkv", bufs=2, space="PSUM")
                ps_qkv = ps_qkv_cm.__enter__()
                ps_y_cur = [ps_y]

                def qk_chain(f8, t4):
                    ps = ps_qkv.tile([128, TQ], F32, tag="qkv",
                                     name=f"qkch_{f8}_{t4}")
                    for e in range(EC):
                        nc.tensor.matmul(
                            ps[:], wqk_sb[e][:, f8 * 128:(f8 + 1) * 128],
                            xt_sb[(t4, e)][:],
                            start=(e == 0), stop=(e == EC - 1),
                            skip_group_check=True,
                        )
                    rope_evict(ps, f8, t4)

                def v_chain(i, t4):
                    tk = 4 * t4 + i
                    ps = ps_qkv.tile([128, FL], F32, tag="qkv",
                                     name=f"vch_{tk}")
                    for e in range(EC):
                        nc.tensor.matmul(
                            ps[:], xt_sb[(t4, e)][:, i * 128:(i + 1) * 128],
                            wv_sb[e][:],
                            start=(e == 0), stop=(e == EC - 1),
                            skip_group_check=True,
                        )
                    nc.scalar.copy(v_sb[tk][:], ps[:])

                def proj_chunk(tqb, alt=False):
                    # out rows tqb*128..+128, partial over local heads
                    for nb in range(NTQ):
                        ps = ps_o.tile([128, TQ], F32, tag="o",
                                       name=f"ops_{tqb}_{nb}")
                        for h in range(HPC):
                            nc.tensor.matmul(
                                ps[:],
                                yt_sb[h][tqb // 4][:, (tqb % 4) * 128:
                                                   (tqb % 4 + 1) * 128],
                                wo_sb[h][:, nb * TQ:(nb + 1) * TQ],
                                start=(h == 0), stop=(h == HPC - 1),
                                skip_group_check=True,
                            )
                        osb = o_ev.tile([128, TQ], F16, tag="osb",
                                        name=f"osb_{tqb}_{nb}")
                        if alt and nb % 2 == 1:
                            nc.vector.tensor_copy(osb[:], ps[:])
                        else:
                            nc.scalar.copy(osb[:], ps[:])
                        nc.sync.dma_start(
                            out[tqb * 128:(tqb + 1) * 128,
                                nb * TQ:(nb + 1) * TQ], osb[:])

                def pop(fillers, n=1):
                    for _ in range(n):
                        if fillers:
                            fillers.pop(0)()

                def sdpa_head(h, tq, fillers, drain=False, nfill=1):
                    nblk = 4 * tq + 4
                    ngrp = tq + 1
                    qs = qkt_sb[h][tq]
                    yps = ps_y_cur[0].tile([128, TQ], F32, tag="y",
                                           name=f"yps_{h}_{tq}")
                    es_of = {}
                    acc = None

                    def scores_grp(g):
                        for r4 in range(4):
                            tk = 4 * g + r4
                            dr = tk - 4 * tq
                            lo = 128 * dr if dr > 0 else 0
                            sps = ps_sc.tile([128, TQ], F32, tag="sc",
                                             name=f"sps_{h}_{tq}_{tk}")
                            nc.tensor.matmul(
                                sps[:, lo:TQ],
                                qkt_sb[HPC + h][tk // 4][:, (tk % 4) * 128:
                                                         (tk % 4 + 1) * 128],
                                qs[:, lo:TQ], start=True, stop=True,
                                skip_group_check=True,
                            )
                            es = es_p.tile([128, TQ], F16, tag=f"es{r4}",
                                           name=f"es_{h}_{tq}_{tk}")
                            nc.scalar.activation(es[:, lo:TQ], sps[:, lo:TQ],
                                                 EXP, scale=SCALE)
                            if dr >= 0:
                                nc.vector.tensor_tensor(
                                    es[:, lo:lo + 128], es[:, lo:lo + 128],
                                    mask_sb[:], op=AluOpType.mult)
                            es_of[tk] = es

                    def av_grp(g):
                        for r4 in range(4):
                            tk = 4 * g + r4
                            dr = tk - 4 * tq
                            lo = 128 * dr if dr > 0 else 0
                            nc.tensor.matmul(
                                yps[:, lo:TQ],
                                v_sb[tk][:, h * 128:(h + 1) * 128],
                                es_of[tk][:, lo:TQ],
                                start=(tk == 0), stop=(tk == nblk - 1),
                                skip_group_check=True,
                            )

                    def dsum_grp(g):
                        # fp16 sum tree of the 4 (full) blocks of group g
                        nonlocal acc
                        e0, e1, e2, e3 = (es_of[4 * g + r] for r in range(4))
                        pra = dn_p.tile([128, TQ], F16, tag="pra")
                        prb = dn_p.tile([128, TQ], F16, tag="prb")
                        nc.vector.tensor_tensor(pra[:], e0[:], e1[:],
                                                op=AluOpType.add)
                        nc.vector.tensor_tensor(prb[:], e2[:], e3[:],
                                                op=AluOpType.add)
                        if acc is None:
                            acc = dn_p.tile([128, TQ], F16, tag="acc")
                            nc.vector.tensor_tensor(acc[:], pra[:], prb[:],
                                                    op=AluOpType.add)
                        else:
                            gs = dn_p.tile([128, TQ], F16, tag="gs", bufs=1)
                            nc.vector.tensor_tensor(gs[:], pra[:], prb[:],
                                                    op=AluOpType.add)
                            nc.vector.tensor_tensor(acc[:], acc[:], gs[:],
                                                    op=AluOpType.add)

                    def dsum_diag():
                        # ragged sum of the diagonal band into a fresh tile
                        # (read-only on es tiles, so the all_reduce below can
                        # overlap the diagonal AV matmuls)
                        nonlocal acc
                        d0, d1, d2, d3 = (es_of[4 * tq + r] for r in range(4))
                        prd = dn_p.tile([128, TQ], F16, tag="prd", bufs=1)
                        nc.vector.tensor_copy(prd[:, 0:128], d0[:, 0:128])
                        nc.vector.tensor_tensor(prd[:, 128:TQ],
                                                d0[:, 128:TQ], d1[:, 128:TQ],
                                                op=AluOpType.add)
                        nc.vector.tensor_tensor(prd[:, 256:TQ],
                                                prd[:, 256:TQ], d2[:, 256:TQ],
                                                op=AluOpType.add)
                        nc.vector.tensor_tensor(prd[:, 384:TQ],
                                                prd[:, 384:TQ], d3[:, 384:TQ],
                                                op=AluOpType.add)
                        if acc is None:
                            return prd
                        nc.vector.tensor_tensor(acc[:], acc[:], prd[:],
                                                op=AluOpType.add)
                        return acc

                    for g in range(ngrp):
                        scores_grp(g)
                        if g > 0:
                            av_grp(g - 1)
                            dsum_grp(g - 1)
                        pop(fillers, nfill)
                    dacc = dsum_diag()
                    rcp = dn_p.tile([128, TQ], F32, tag="rcp",
                                    name=f"rcp_{h}_{tq}")
                    if (h, tq) == (HPC - 1, NTQ - 1):
                        # final head gates the tail: use the short
                        # ones-matmul + broadcast chain instead of the
                        # ~3.5us gpsimd all_reduce
                        dps = ps_y_cur[0].tile([1, TQ], F32, tag="y",
                                               name="dps_tail")
                        nc.tensor.matmul(dps[:], ones_sb[:], dacc[:],
                                         start=True, stop=True,
                                         skip_group_check=True)
                        rrow = dn_p.tile([1, TQ], F32, tag="rrow", bufs=1)
                        nc.vector.reciprocal_approx_fast(rrow[:], dps[:])
                        nc.gpsimd.partition_broadcast(rcp[:], rrow[:])
                    else:
                        rb = dn_p.tile([128, TQ], F32, tag="rb",
                                       name=f"rb_{h}_{tq}")
                        nc.gpsimd.partition_all_reduce(
                            rb[:], dacc[:], channels=128,
                            reduce_op=bass_isa.ReduceOp.add)
                        nc.vector.reciprocal_approx_fast(rcp[:], rb[:])
                    av_grp(ngrp - 1)
                    pop(fillers, nfill)
                    nc.vector.tensor_tensor(yt_sb[h][tq][:], yps[:], rcp[:],
                                            op=AluOpType.mult)
                    if drain:
                        while fillers:
                            pop(fillers)

                # QKV filler order: KT chains (f8 4..7) interleaved with QT
                # (f8 0..3) so the next slot's first score matmuls find both
                # its KT and the early heads' QT already RoPE'd; V last (its
                # consumers are each head's final AV group next slot).
                def qkv_fillers(t4):
                    fs = []
                    for k in range(4):
                        fs.append(lambda f8=4 + k, t=t4: qk_chain(f8, t))
                        fs.append(lambda f8=k, t=t4: qk_chain(f8, t))
                    fs += [lambda i=i, t=t4: v_chain(i, t) for i in range(4)]
                    return fs

                # slot 1: sdpa tq=0, filler = QKV t4=1
                fillers = qkv_fillers(1)
                for h in range(HPC):
                    sdpa_head(h, 0, fillers, drain=(h == HPC - 1), nfill=2)
                dma_x_slab(3)

                # slot 2: sdpa tq=1, filler = QKV t4=2 AND t4=3 (pulled
                # forward so the exp-heavy tq=3 heads can spread over two
                # slots below)
                fillers = qkv_fillers(2) + qkv_fillers(3)
                for h in range(HPC):
                    sdpa_head(h, 1, fillers, drain=(h == HPC - 1), nfill=2)

                ps_qkv_cm.__exit__(None, None, None)
                ps_y34_cm = tc.tile_pool(name="ps_y34", bufs=2, space="PSUM")
                ps_y_cur[0] = ps_y34_cm.__enter__()

                # slot 3: sdpa tq=2 (4 heads) + sdpa tq=3 (heads 0,1),
                # filler = proj 0..7 rationed per head
                proj_fill = [lambda t=t: proj_chunk(t) for t in range(0, 8)]
                for h in range(HPC):
                    sdpa_head(h, 2, [proj_fill.pop(0)])
                sdpa_head(0, 3, [proj_fill.pop(0) for _ in range(2)])
                sdpa_head(1, 3, [proj_fill.pop(0) for _ in range(2)])

                # slot 4: sdpa tq=3 heads 2,3 with proj 8..11 rationed;
                # tail = proj 12..15
                proj_fill = [lambda t=t: proj_chunk(t) for t in range(8, 12)]
                sdpa_head(2, 3, [proj_fill.pop(0) for _ in range(3)])
                # final head runs with a single filler; its remaining proj
                # chunk executes after the chain, covering the denominator
                # tail so proj 12..15 start the moment yt[3][3] lands
                sdpa_head(3, 3, [proj_fill.pop(0)])
                while proj_fill:
                    pop(proj_fill)
                for t in range(12, 16):
                    proj_chunk(t, alt=True)
                ps_y34_cm.__exit__(None, None, None)

    nc.compile()
    return nc


def _host_tables():
    positions = np.arange(T, dtype=np.float64)
    inv_freq = 1.0 / (THETA ** (np.arange(0, D, 2, dtype=np.float64) / D))
    freqs = np.outer(positions, inv_freq)          # [T, 64]
    cs = np.concatenate([np.cos(freqs).T, np.sin(freqs).T]).astype(np.float16)
    cs2 = np.concatenate([np.sin(freqs).T, np.cos(freqs).T]).astype(np.float16)
    p = np.arange(128)[:, None]
    j = np.arange(128)[None, :]
    mask = (p <= j).astype(np.float16)             # [128, 128] triangle
    return cs, cs2, mask


def kernel(x, W_qkv, W_out):
    global _compiled
    if _compiled is None:
        _compiled = _build()
    nc = _compiled

    x = np.ascontiguousarray(np.asarray(x, dtype=np.float32))
    W_qkv = np.asarray(W_qkv, dtype=np.float32)
    W_out = np.asarray(W_out, dtype=np.float32)

    cs, cs2, mask = _host_tables()

    perm = np.concatenate([np.arange(0, D, 2), np.arange(1, D, 2)])  # de-interleave

    in_maps = []
    for c in range(N_CORES):
        b, tp = divmod(c, TP)
        heads = np.arange(tp * HPC, (tp + 1) * HPC)
        qk_cols = np.concatenate(
            [h * D + perm for h in heads] + [E + h * D + perm for h in heads]
        )
        v_cols = np.concatenate([2 * E + h * D + np.arange(D) for h in heads])
        wqk_l = np.ascontiguousarray(W_qkv[:, qk_cols]).reshape(EC, 128, 2 * FL)
        wv_l = np.ascontiguousarray(W_qkv[:, v_cols]).reshape(EC, 128, FL)
        wout_l = np.ascontiguousarray(
            W_out.reshape(N_HEAD, D, E)[heads].reshape(HPC, 128, E)
        )
        in_maps.append({
            "xT": np.ascontiguousarray(x[b].T).astype(np.float16),
            "wqk": wqk_l.astype(np.float16),
            "wv": wv_l.astype(np.float16),
            "wout": wout_l.astype(np.float16),
            "csx": cs,
            "csx2": cs2,
            "maskx": mask,
            "onesx": np.ones((128, 1), np.float16),
        })

    global _last_in_maps
    _last_in_maps = in_maps
    res = bass_utils.run_bass_kernel_spmd(nc, in_maps, core_ids=list(range(N_CORES)))
    out = np.zeros((B, T, E), dtype=np.float32)
    for c in range(N_CORES):
        out[c // TP] += res.results[c]["out"].astype(np.float32)
    return out


# revision 32
# speedup vs baseline: 1.0097x; 1.0097x over previous
"""Trainium2 Bass kernel for a causal attention block (B=2, T=2048, E=2048,
16 heads, head_dim=128, interleaved RoPE).

Sharding: data-parallel over batch (2) x tensor-parallel over heads (4 per
core) = 8 NeuronCores. Each core computes QKV projection for its 4 heads,
RoPE, causal SDPA, and a partial output projection (row-sharded W_out); the
host sums the 4 TP partials per batch element.

Single fully-pipelined instruction stream (fp16 matmul inputs, fp32 PSUM):
  slot 0:  QKV for t4=0 with e-major accumulation (PE starts as soon as the
           first x/W chunk DMA lands; 8 QK PSUM banks live).
  slot t (1..3): QKV chains for t4=t are woven as PE filler between the
           SDPA score/AV groups of tq=t-1, so ACT (exp) latency never stalls
           the PE. Out-projection chunks for tq=t-2 join the filler pool.
  slot 4:  SDPA tq=3 with proj chunks 8..11 as filler, then proj 12..15.

SDPA is causally trimmed at 128-granularity: diagonal-band blocks compute
only the valid trailing query range (53.1% of full score work instead of
62.5% with 512-wide tiles). Softmax denominators: fp16 tile-sum tree on DVE,
partition_all_reduce on gpsimd (no PE ones-matmuls, no separate broadcast),
fast reciprocal + normalize on DVE.
"""

import sys

sys.path.insert(0, "/opt/trn_rl_repo")

import numpy as np

import concourse.bacc as bacc
import concourse.mybir as mybir
from concourse.alu_op_type import AluOpType
from concourse import bass_isa
from concourse import tile
from concourse import bass_utils

B, T, E = 2, 2048, 2048
N_HEAD = 16
D = E // N_HEAD            # 128
THETA = 10000.0
N_CORES = 8
TP = 4                     # tensor-parallel degree (heads)
HPC = N_HEAD // TP         # heads per core = 4
FL = HPC * D               # local head width = 512
EC = E // 128              # 16 contraction chunks
TQ = 512                   # query tile (free dim)
NTQ = T // TQ              # 4
NTK = T // 128             # 16

F32 = mybir.dt.float32
F16 = mybir.dt.float16
EXP = mybir.ActivationFunctionType.Exp
SCALE = 1.0 / np.sqrt(D)

_compiled = None
_last_in_maps = None


def _build():
    nc = bacc.Bacc("TRN2", target_bir_lowering=False)

    xT = nc.dram_tensor("xT", (E, T), F16, kind="ExternalInput")
    wqk = nc.dram_tensor("wqk", (EC, 128, 2 * FL), F16, kind="ExternalInput")
    wv = nc.dram_tensor("wv", (EC, 128, FL), F16, kind="ExternalInput")
    wout = nc.dram_tensor("wout", (HPC, 128, E), F16, kind="ExternalInput")
    csx = nc.dram_tensor("csx", (128, T), F16, kind="ExternalInput")
    csx2 = nc.dram_tensor("csx2", (128, T), F16, kind="ExternalInput")
    maskx = nc.dram_tensor("maskx", (128, 128), F16, kind="ExternalInput")
    onesx = nc.dram_tensor("onesx", (128, 1), F16, kind="ExternalInput")
    out = nc.dram_tensor("out", (T, E), F16, kind="ExternalOutput")

    with tile.TileContext(nc) as tc, nc.allow_low_precision(
        reason="fp16 matmul inputs / fp16 softmax stats are intentional"
    ):
        with tc.tile_pool(name="const", bufs=1) as const, \
             tc.tile_pool(name="qkt_p", bufs=1) as qkt_p, \
             tc.tile_pool(name="v_p", bufs=1) as v_p, \
             tc.tile_pool(name="yt_p", bufs=1) as yt_p, \
             tc.tile_pool(name="w_p", bufs=1) as w_p, \
             tc.tile_pool(name="xt_p", bufs=2) as xt_p, \
             tc.tile_pool(name="rope_p", bufs=2) as rope_p, \
             tc.tile_pool(name="es_p", bufs=2) as es_p, \
             tc.tile_pool(name="dn_p", bufs=2) as dn_p, \
             tc.tile_pool(name="o_ev", bufs=4) as o_ev:

            cs_sb = const.tile([128, T], F16, tag="cs")    # [cos; sin]
            csd_sb = const.tile([128, T], F16, tag="csd")  # [sin; cos]
            mask_sb = const.tile([128, 128], F16, tag="mask")
            ones_sb = const.tile([128, 1], F16, tag="ones")

            qkt_sb = [[qkt_p.tile([128, TQ], F16, tag=f"qkt{f}_{t}",
                                  name=f"qkt_sb{f}_{t}")
                       for t in range(NTQ)] for f in range(2 * HPC)]
            v_sb = [v_p.tile([128, FL], F16, tag=f"v{t}", name=f"v_sb{t}")
                    for t in range(NTK)]
            yt_sb = [[yt_p.tile([128, TQ], F16, tag=f"yt{h}_{t}",
                                name=f"yt_sb{h}_{t}")
                      for t in range(NTQ)] for h in range(HPC)]
            wqk_sb = [w_p.tile([128, 2 * FL], F16, tag=f"wqk{e}",
                               name=f"wqk_sb{e}") for e in range(EC)]
            wv_sb = [w_p.tile([128, FL], F16, tag=f"wv{e}", name=f"wv_sb{e}")
                     for e in range(EC)]
            wo_sb = [w_p.tile([128, E], F16, tag=f"wo{h}", name=f"wo_sb{h}")
                     for h in range(HPC)]

            xt_sb = {}  # (t4, e) -> tile

            def dma_x_slab(t4):
                for e in range(EC):
                    xt = xt_p.tile([128, TQ], F16, tag=f"xt{e}")
                    nc.sync.dma_start(
                        xt[:], xT[e * 128:(e + 1) * 128,
                                  t4 * TQ:(t4 + 1) * TQ])
                    xt_sb[(t4, e)] = xt

            def rope_evict(ps, f8, t4):
                # PSUM [128,TQ] f32 -> fp16 copy -> RoPE rotation on DVE
                # (all-fp16 SBUF operands; inputs pairwise share a base
                # partition as the ALU engines require)
                ts4 = slice(t4 * TQ, (t4 + 1) * TQ)
                qk16 = rope_p.tile([128, TQ], F16, tag="qk16")
                nc.scalar.copy(qk16[:], ps[:])
                dst = qkt_sb[f8][t4]
                t_a = rope_p.tile([64, TQ], F16, tag="ta", bufs=1)
                t_b = rope_p.tile([64, TQ], F16, tag="tb", bufs=1)
                nc.vector.tensor_tensor(t_a[:], qk16[0:64, :],
                                        cs_sb[0:64, ts4], op=AluOpType.mult)
                nc.vector.tensor_tensor(t_b[:], qk16[64:128, :],
                                        cs_sb[64:128, ts4], op=AluOpType.mult)
                nc.vector.tensor_tensor(dst[0:64, :], t_a[:], t_b[:],
                                        op=AluOpType.subtract)
                t_c = rope_p.tile([64, TQ], F16, tag="tc", bufs=1)
                t_d = rope_p.tile([64, TQ], F16, tag="td", bufs=1)
                nc.vector.tensor_tensor(t_c[:], qk16[0:64, :],
                                        csd_sb[0:64, ts4], op=AluOpType.mult)
                nc.vector.tensor_tensor(t_d[:], qk16[64:128, :],
                                        csd_sb[64:128, ts4], op=AluOpType.mult)
                nc.vector.tensor_tensor(dst[64:128, :], t_c[:], t_d[:],
                                        op=AluOpType.add)

            # ---------------- DMA preamble (priority order) ----------------
            # dma_start issue costs ~0.6us on the sync queue (8 outstanding),
            # so keep the count low and issue the first compute's chunks
            # before anything else.
            for e in range(8):
                xt = xt_p.tile([128, TQ], F16, tag=f"xt{e}")
                if e == 0:
                    nc.sync.dma_start(xt[:, 0:256], xT[0:128, 0:256])
                    nc.sync.dma_start(xt[:, 256:TQ], xT[0:128, 256:TQ])
                    nc.sync.dma_start(wqk_sb[0][:, 0:FL], wqk[0][:, 0:FL])
                    nc.sync.dma_start(wqk_sb[0][:, FL:2 * FL],
                                      wqk[0][:, FL:2 * FL])
                else:
                    nc.sync.dma_start(xt[:],
                                      xT[e * 128:(e + 1) * 128, 0:TQ])
                    nc.sync.dma_start(wqk_sb[e][:], wqk[e])
                xt_sb[(0, e)] = xt
            nc.sync.dma_start(cs_sb[:], csx[:])
            nc.sync.dma_start(csd_sb[:], csx2[:])
            nc.sync.dma_start(mask_sb[:], maskx[:])
            nc.sync.dma_start(ones_sb[:], onesx[:])
            for e in range(8, EC):
                xt = xt_p.tile([128, TQ], F16, tag=f"xt{e}")
                nc.sync.dma_start(xt[:], xT[e * 128:(e + 1) * 128, 0:TQ])
                xt_sb[(0, e)] = xt
                nc.sync.dma_start(wqk_sb[e][:], wqk[e])
            for e in range(EC):
                nc.sync.dma_start(wv_sb[e][:], wv[e])

            # ---------------- slot 0: QKV for t4=0, e-major ----------------
            # Two e-major half-passes (f8 0..3 then 4..7) so the RoPE
            # evictions of each half drain on ACT/DVE under the next pass's
            # matmuls instead of bunching at a single loop end.
            with tc.tile_pool(name="ps0", bufs=1, space="PSUM") as ps0:
                qkps = [ps0.tile([128, TQ], F32, tag=f"qk{f8}",
                                 name=f"qkps{f8}") for f8 in range(2 * HPC)]
                for e in range(EC):
                    for f8 in range(HPC):
                        nc.tensor.matmul(
                            qkps[f8][:],
                            wqk_sb[e][:, f8 * 128:(f8 + 1) * 128],
                            xt_sb[(0, e)][:],
                            start=(e == 0), stop=(e == EC - 1),
                            skip_group_check=True,
                        )
                # prefetch next x slab + wout while evictions/V run
                dma_x_slab(1)
                for h in range(HPC):
                    nc.sync.dma_start(wo_sb[h][:], wout[h])
                for f8 in range(HPC):
                    rope_evict(qkps[f8], f8, 0)
                for e in range(EC):
                    for f8 in range(HPC, 2 * HPC):
                        nc.tensor.matmul(
                            qkps[f8][:],
                            wqk_sb[e][:, f8 * 128:(f8 + 1) * 128],
                            xt_sb[(0, e)][:],
                            start=(e == 0), stop=(e == EC - 1),
                            skip_group_check=True,
                        )
                for f8 in range(HPC, 2 * HPC):
                    rope_evict(qkps[f8], f8, 0)
                for i in range(4):
                    ps = ps0.tile([128, FL], F32, tag=f"qk{i}",
                                  name=f"vps{i}")
                    for e in range(EC):
                        nc.tensor.matmul(
                            ps[:], xt_sb[(0, e)][:, i * 128:(i + 1) * 128],
                            wv_sb[e][:],
                            start=(e == 0), stop=(e == EC - 1),
                            skip_group_check=True,
                        )
                    nc.scalar.copy(v_sb[i][:], ps[:])
                dma_x_slab(2)

            # ---------------- slots 1..4: pipelined SDPA ----------------
            # ps_qkv (2 banks) lives only through slot 2; its banks are then
            # recycled into a 2-deep yps ring for slots 3-4 so a head's first
            # AV matmul never waits the previous head's normalize (DVE).
            with tc.tile_pool(name="ps_sc", bufs=3, space="PSUM") as ps_sc, \
                 tc.tile_pool(name="ps_y", bufs=1, space="PSUM") as ps_y, \
                 tc.tile_pool(name="ps_o", bufs=2, space="PSUM") as ps_o:
                ps_qkv_cm = tc.tile_pool(name="ps_qkv", bufs=2, space="PSUM")
                ps_qkv = ps_qkv_cm.__enter__()
                ps_y_cur = [ps_y]

                def qk_chain(f8, t4):
                    ps = ps_qkv.tile([128, TQ], F32, tag="qkv",
                                     name=f"qkch_{f8}_{t4}")
                    for e in range(EC):
                        nc.tensor.matmul(
                            ps[:], wqk_sb[e][:, f8 * 128:(f8 + 1) * 128],
                            xt_sb[(t4, e)][:],
                            start=(e == 0), stop=(e == EC - 1),
                            skip_group_check=True,
                        )
                    rope_evict(ps, f8, t4)

                def v_chain(i, t4):
                    tk = 4 * t4 + i
                    ps = ps_qkv.tile([128, FL], F32, tag="qkv",
                                     name=f"vch_{tk}")
                    for e in range(EC):
                        nc.tensor.matmul(
                            ps[:], xt_sb[(t4, e)][:, i * 128:(i + 1) * 128],
                            wv_sb[e][:],
                            start=(e == 0), stop=(e == EC - 1),
                            skip_group_check=True,
                        )
                    nc.scalar.copy(v_sb[tk][:], ps[:])

                def proj_unit(tqb, nb, alt=False, split=False):
                    ps = ps_o.tile([128, TQ], F32, tag="o",
                                   name=f"ops_{tqb}_{nb}")
                    for h in range(HPC):
                        nc.tensor.matmul(
                            ps[:],
                            yt_sb[h][tqb // 4][:, (tqb % 4) * 128:
                                               (tqb % 4 + 1) * 128],
                            wo_sb[h][:, nb * TQ:(nb + 1) * TQ],
                            start=(h == 0), stop=(h == HPC - 1),
                            skip_group_check=True,
                        )
                    osb = o_ev.tile([128, TQ], F16, tag="osb",
                                    name=f"osb_{tqb}_{nb}")
                    if split:
                        # halves on both engines in parallel for the final
                        # flush latency
                        nc.scalar.copy(osb[:, 0:256], ps[:, 0:256])
                        nc.vector.tensor_copy(osb[:, 256:TQ], ps[:, 256:TQ])
                        for p in range(2):
                            c0 = nb * TQ + p * 256
                            nc.sync.dma_start(
                                out[tqb * 128:(tqb + 1) * 128, c0:c0 + 256],
                                osb[:, p * 256:(p + 1) * 256])
                        return
                    if alt and nb % 2 == 1:
                        nc.vector.tensor_copy(osb[:], ps[:])
                    else:
                        nc.scalar.copy(osb[:], ps[:])
                    nc.sync.dma_start(
                        out[tqb * 128:(tqb + 1) * 128,
                            nb * TQ:(nb + 1) * TQ], osb[:])

                def proj_chunk(tqb, alt=False):
                    # out rows tqb*128..+128, partial over local heads
                    for nb in range(NTQ):
                        proj_unit(tqb, nb, alt)

                def pop(fillers, n=1):
                    for _ in range(n):
                        if fillers:
                            fillers.pop(0)()

                def sdpa_head(h, tq, fillers, drain=False, nfill=1):
                    nblk = 4 * tq + 4
                    ngrp = tq + 1
                    qs = qkt_sb[h][tq]
                    yps = ps_y_cur[0].tile([128, TQ], F32, tag="y",
                                           name=f"yps_{h}_{tq}")
                    es_of = {}
                    acc = None

                    def scores_grp(g):
                        for r4 in range(4):
                            tk = 4 * g + r4
                            dr = tk - 4 * tq
                            lo = 128 * dr if dr > 0 else 0
                            sps = ps_sc.tile([128, TQ], F32, tag="sc",
                                             name=f"sps_{h}_{tq}_{tk}")
                            nc.tensor.matmul(
                                sps[:, lo:TQ],
                                qkt_sb[HPC + h][tk // 4][:, (tk % 4) * 128:
                                                         (tk % 4 + 1) * 128],
                                qs[:, lo:TQ], start=True, stop=True,
                                skip_group_check=True,
                            )
                            es = es_p.tile([128, TQ], F16, tag=f"es{r4}",
                                           name=f"es_{h}_{tq}_{tk}")
                            nc.scalar.activation(es[:, lo:TQ], sps[:, lo:TQ],
                                                 EXP, scale=SCALE)
                            if dr >= 0:
                                nc.vector.tensor_tensor(
                                    es[:, lo:lo + 128], es[:, lo:lo + 128],
                                    mask_sb[:], op=AluOpType.mult)
                            es_of[tk] = es

                    def av_grp(g):
                        for r4 in range(4):
                            tk = 4 * g + r4
                            dr = tk - 4 * tq
                            lo = 128 * dr if dr > 0 else 0
                            nc.tensor.matmul(
                                yps[:, lo:TQ],
                                v_sb[tk][:, h * 128:(h + 1) * 128],
                                es_of[tk][:, lo:TQ],
                                start=(tk == 0), stop=(tk == nblk - 1),
                                skip_group_check=True,
                            )

                    def dsum_grp(g):
                        # fp16 sum tree of the 4 (full) blocks of group g
                        nonlocal acc
                        e0, e1, e2, e3 = (es_of[4 * g + r] for r in range(4))
                        pra = dn_p.tile([128, TQ], F16, tag="pra")
                        prb = dn_p.tile([128, TQ], F16, tag="prb")
                        nc.vector.tensor_tensor(pra[:], e0[:], e1[:],
                                                op=AluOpType.add)
                        nc.vector.tensor_tensor(prb[:], e2[:], e3[:],
                                                op=AluOpType.add)
                        if acc is None:
                            acc = dn_p.tile([128, TQ], F16, tag="acc")
                            nc.vector.tensor_tensor(acc[:], pra[:], prb[:],
                                                    op=AluOpType.add)
                        else:
                            gs = dn_p.tile([128, TQ], F16, tag="gs", bufs=1)
                            nc.vector.tensor_tensor(gs[:], pra[:], prb[:],
                                                    op=AluOpType.add)
                            nc.vector.tensor_tensor(acc[:], acc[:], gs[:],
                                                    op=AluOpType.add)

                    def dsum_diag():
                        # ragged sum of the diagonal band into a fresh tile
                        # (read-only on es tiles, so the all_reduce below can
                        # overlap the diagonal AV matmuls)
                        nonlocal acc
                        d0, d1, d2, d3 = (es_of[4 * tq + r] for r in range(4))
                        prd = dn_p.tile([128, TQ], F16, tag="prd", bufs=1)
                        nc.vector.tensor_copy(prd[:, 0:128], d0[:, 0:128])
                        nc.vector.tensor_tensor(prd[:, 128:TQ],
                                                d0[:, 128:TQ], d1[:, 128:TQ],
                                                op=AluOpType.add)
                        nc.vector.tensor_tensor(prd[:, 256:TQ],
                                                prd[:, 256:TQ], d2[:, 256:TQ],
                                                op=AluOpType.add)
                        nc.vector.tensor_tensor(prd[:, 384:TQ],
                                                prd[:, 384:TQ], d3[:, 384:TQ],
                                                op=AluOpType.add)
                        if acc is None:
                            return prd
                        nc.vector.tensor_tensor(acc[:], acc[:], prd[:],
                                                op=AluOpType.add)
                        return acc

                    for g in range(ngrp):
                        scores_grp(g)
                        if g > 0:
                            av_grp(g - 1)
                            dsum_grp(g - 1)
                        pop(fillers, nfill)
                    dacc = dsum_diag()
                    rcp = dn_p.tile([128, TQ], F32, tag="rcp",
                                    name=f"rcp_{h}_{tq}")
                    if (h, tq) == (HPC - 1, NTQ - 1):
                        # final head gates the tail: use the short
                        # ones-matmul + broadcast chain instead of the
                        # ~3.5us gpsimd all_reduce
                        dps = ps_y_cur[0].tile([1, TQ], F32, tag="y",
                                               name="dps_tail")
                        nc.tensor.matmul(dps[:], ones_sb[:], dacc[:],
                                         start=True, stop=True,
                                         skip_group_check=True)
                        rrow = dn_p.tile([1, TQ], F32, tag="rrow", bufs=1)
                        nc.vector.reciprocal_approx_fast(rrow[:], dps[:])
                        nc.gpsimd.partition_broadcast(rcp[:], rrow[:])
                    else:
                        rb = dn_p.tile([128, TQ], F32, tag="rb",
                                       name=f"rb_{h}_{tq}")
                        nc.gpsimd.partition_all_reduce(
                            rb[:], dacc[:], channels=128,
                            reduce_op=bass_isa.ReduceOp.add)
                        nc.vector.reciprocal_approx_fast(rcp[:], rb[:])
                    av_grp(ngrp - 1)
                    pop(fillers, nfill)
                    nc.vector.tensor_tensor(yt_sb[h][tq][:], yps[:], rcp[:],
                                            op=AluOpType.mult)
                    if drain:
                        while fillers:
                            pop(fillers)

                # QKV filler order: KT chains (f8 4..7) interleaved with QT
                # (f8 0..3) so the next slot's first score matmuls find both
                # its KT and the early heads' QT already RoPE'd; V last (its
                # consumers are each head's final AV group next slot).
                def qkv_fillers(t4):
                    fs = []
                    for k in range(4):
                        fs.append(lambda f8=4 + k, t=t4: qk_chain(f8, t))
                        fs.append(lambda f8=k, t=t4: qk_chain(f8, t))
                    fs += [lambda i=i, t=t4: v_chain(i, t) for i in range(4)]
                    return fs

                # slot 1: sdpa tq=0, filler = QKV t4=1
                fillers = qkv_fillers(1)
                for h in range(HPC):
                    sdpa_head(h, 0, fillers, drain=(h == HPC - 1), nfill=2)
                dma_x_slab(3)

                # slot 2: sdpa tq=1, filler = QKV t4=2 AND t4=3 (pulled
                # forward so the exp-heavy tq=3 heads can spread over two
                # slots below)
                fillers = qkv_fillers(2) + qkv_fillers(3)
                for h in range(HPC):
                    sdpa_head(h, 1, fillers, drain=(h == HPC - 1), nfill=2)

                ps_qkv_cm.__exit__(None, None, None)
                ps_y34_cm = tc.tile_pool(name="ps_y34", bufs=2, space="PSUM")
                ps_y_cur[0] = ps_y34_cm.__enter__()

                # slot 3: sdpa tq=2 (4 heads) + sdpa tq=3 (heads 0,1),
                # filler = proj 0..7 rationed per head
                proj_fill = [lambda t=t: proj_chunk(t) for t in range(0, 8)]
                for h in range(HPC):
                    sdpa_head(h, 2, [proj_fill.pop(0)])
                sdpa_head(0, 3, [proj_fill.pop(0) for _ in range(2)])
                sdpa_head(1, 3, [proj_fill.pop(0) for _ in range(2)])

                # slot 4: sdpa tq=3 heads 2,3 with proj 8..11 rationed;
                # tail = proj 12..15
                sdpa_head(2, 3, [lambda t=t: proj_chunk(t)
                                 for t in range(8, 10)])
                # final head: one proj unit per group boundary keeps the exp
                # chase covered at fine grain, and a full chunk lands right
                # after its last AV to cover the denominator chain
                h3_fill = [lambda nb=nb: proj_unit(10, nb) for nb in range(4)]
                h3_fill.append(lambda: proj_chunk(11))
                sdpa_head(3, 3, h3_fill)
                for t in range(12, 15):
                    proj_chunk(t, alt=True)
                for nb in range(NTQ):
                    proj_unit(15, nb, split=(nb >= 2))
                ps_y34_cm.__exit__(None, None, None)

    nc.compile()
    return nc


def _host_tables():
    positions = np.arange(T, dtype=np.float64)
    inv_freq = 1.0 / (THETA ** (np.arange(0, D, 2, dtype=np.float64) / D))
    freqs = np.outer(positions, inv_freq)          # [T, 64]
    cs = np.concatenate([np.cos(freqs).T, np.sin(freqs).T]).astype(np.float16)
    cs2 = np.concatenate([np.sin(freqs).T, np.cos(freqs).T]).astype(np.float16)
    p = np.arange(128)[:, None]
    j = np.arange(128)[None, :]
    mask = (p <= j).astype(np.float16)             # [128, 128] triangle
    return cs, cs2, mask


def kernel(x, W_qkv, W_out):
    global _compiled
    if _compiled is None:
        _compiled = _build()
    nc = _compiled

    x = np.ascontiguousarray(np.asarray(x, dtype=np.float32))
    W_qkv = np.asarray(W_qkv, dtype=np.float32)
    W_out = np.asarray(W_out, dtype=np.float32)

    cs, cs2, mask = _host_tables()

    perm = np.concatenate([np.arange(0, D, 2), np.arange(1, D, 2)])  # de-interleave

    in_maps = []
    for c in range(N_CORES):
        b, tp = divmod(c, TP)
        heads = np.arange(tp * HPC, (tp + 1) * HPC)
        qk_cols = np.concatenate(
            [h * D + perm for h in heads] + [E + h * D + perm for h in heads]
        )
        v_cols = np.concatenate([2 * E + h * D + np.arange(D) for h in heads])
        wqk_l = np.ascontiguousarray(W_qkv[:, qk_cols]).reshape(EC, 128, 2 * FL)
        wv_l = np.ascontiguousarray(W_qkv[:, v_cols]).reshape(EC, 128, FL)
        wout_l = np.ascontiguousarray(
            W_out.reshape(N_HEAD, D, E)[heads].reshape(HPC, 128, E)
        )
        in_maps.append({
            "xT": np.ascontiguousarray(x[b].T).astype(np.float16),
            "wqk": wqk_l.astype(np.float16),
            "wv": wv_l.astype(np.float16),
            "wout": wout_l.astype(np.float16),
            "csx": cs,
            "csx2": cs2,
            "maskx": mask,
            "onesx": np.ones((128, 1), np.float16),
        })

    global _last_in_maps
    _last_in_maps = in_maps
    res = bass_utils.run_bass_kernel_spmd(nc, in_maps, core_ids=list(range(N_CORES)))
    out = np.zeros((B, T, E), dtype=np.float32)
    for c in range(N_CORES):
        out[c // TP] += res.results[c]["out"].astype(np.float32)
    return out


# revision 33
# speedup vs baseline: 1.0176x; 1.0077x over previous
"""Trainium2 Bass kernel for a causal attention block (B=2, T=2048, E=2048,
16 heads, head_dim=128, interleaved RoPE).

Sharding: data-parallel over batch (2) x tensor-parallel over heads (4 per
core) = 8 NeuronCores. Each core computes QKV projection for its 4 heads,
RoPE, causal SDPA, and a partial output projection (row-sharded W_out); the
host sums the 4 TP partials per batch element.

Single fully-pipelined instruction stream (fp16 matmul inputs, fp32 PSUM):
  slot 0:  QKV for t4=0 with e-major accumulation (PE starts as soon as the
           first x/W chunk DMA lands; 8 QK PSUM banks live).
  slot t (1..3): QKV chains for t4=t are woven as PE filler between the
           SDPA score/AV groups of tq=t-1, so ACT (exp) latency never stalls
           the PE. Out-projection chunks for tq=t-2 join the filler pool.
  slot 4:  SDPA tq=3 with proj chunks 8..11 as filler, then proj 12..15.

SDPA is causally trimmed at 128-granularity: diagonal-band blocks compute
only the valid trailing query range (53.1% of full score work instead of
62.5% with 512-wide tiles). Softmax denominators: fp16 tile-sum tree on DVE,
partition_all_reduce on gpsimd (no PE ones-matmuls, no separate broadcast),
fast reciprocal + normalize on DVE.
"""

import sys

sys.path.insert(0, "/opt/trn_rl_repo")

import numpy as np

import concourse.bacc as bacc
import concourse.mybir as mybir
from concourse.alu_op_type import AluOpType
from concourse import bass_isa
from concourse import tile
from concourse import bass_utils

B, T, E = 2, 2048, 2048
N_HEAD = 16
D = E // N_HEAD            # 128
THETA = 10000.0
N_CORES = 8
TP = 4                     # tensor-parallel degree (heads)
HPC = N_HEAD // TP         # heads per core = 4
FL = HPC * D               # local head width = 512
EC = E // 128              # 16 contraction chunks
TQ = 512                   # query tile (free dim)
NTQ = T // TQ              # 4
NTK = T // 128             # 16

F32 = mybir.dt.float32
F16 = mybir.dt.float16
EXP = mybir.ActivationFunctionType.Exp
SCALE = 1.0 / np.sqrt(D)

_compiled = None
_last_in_maps = None


def _build():
    nc = bacc.Bacc("TRN2", target_bir_lowering=False)

    xT = nc.dram_tensor("xT", (E, T), F16, kind="ExternalInput")
    wqk = nc.dram_tensor("wqk", (EC, 128, 2 * FL), F16, kind="ExternalInput")
    wv = nc.dram_tensor("wv", (EC, 128, FL), F16, kind="ExternalInput")
    wout = nc.dram_tensor("wout", (HPC, 128, E), F16, kind="ExternalInput")
    csx = nc.dram_tensor("csx", (128, T), F16, kind="ExternalInput")
    csx2 = nc.dram_tensor("csx2", (128, T), F16, kind="ExternalInput")
    maskx = nc.dram_tensor("maskx", (128, 128), F16, kind="ExternalInput")
    onesx = nc.dram_tensor("onesx", (128, 1), F16, kind="ExternalInput")
    out = nc.dram_tensor("out", (T, E), F16, kind="ExternalOutput")

    with tile.TileContext(nc) as tc, nc.allow_low_precision(
        reason="fp16 matmul inputs / fp16 softmax stats are intentional"
    ):
        with tc.tile_pool(name="const", bufs=1) as const, \
             tc.tile_pool(name="qkt_p", bufs=1) as qkt_p, \
             tc.tile_pool(name="v_p", bufs=1) as v_p, \
             tc.tile_pool(name="yt_p", bufs=1) as yt_p, \
             tc.tile_pool(name="w_p", bufs=1) as w_p, \
             tc.tile_pool(name="xt_p", bufs=2) as xt_p, \
             tc.tile_pool(name="rope_p", bufs=2) as rope_p, \
             tc.tile_pool(name="es_p", bufs=2) as es_p, \
             tc.tile_pool(name="dn_p", bufs=2) as dn_p, \
             tc.tile_pool(name="o_ev", bufs=4) as o_ev:

            cs_sb = const.tile([128, T], F16, tag="cs")    # [cos; sin]
            csd_sb = const.tile([128, T], F16, tag="csd")  # [sin; cos]
            mask_sb = const.tile([128, 128], F16, tag="mask")
            ones_sb = const.tile([128, 1], F16, tag="ones")

            qkt_sb = [[qkt_p.tile([128, TQ], F16, tag=f"qkt{f}_{t}",
                                  name=f"qkt_sb{f}_{t}")
                       for t in range(NTQ)] for f in range(2 * HPC)]
            v_sb = [v_p.tile([128, FL], F16, tag=f"v{t}", name=f"v_sb{t}")
                    for t in range(NTK)]
            yt_sb = [[yt_p.tile([128, TQ], F16, tag=f"yt{h}_{t}",
                                name=f"yt_sb{h}_{t}")
                      for t in range(NTQ)] for h in range(HPC)]
            wqk_sb = [w_p.tile([128, 2 * FL], F16, tag=f"wqk{e}",
                               name=f"wqk_sb{e}") for e in range(EC)]
            wv_sb = [w_p.tile([128, FL], F16, tag=f"wv{e}", name=f"wv_sb{e}")
                     for e in range(EC)]
            wo_sb = [w_p.tile([128, E], F16, tag=f"wo{h}", name=f"wo_sb{h}")
                     for h in range(HPC)]

            xt_sb = {}  # (t4, e) -> tile

            def dma_x_slab(t4):
                for e in range(EC):
                    xt = xt_p.tile([128, TQ], F16, tag=f"xt{e}")
                    nc.sync.dma_start(
                        xt[:], xT[e * 128:(e + 1) * 128,
                                  t4 * TQ:(t4 + 1) * TQ])
                    xt_sb[(t4, e)] = xt

            def rope_evict(ps, f8, t4):
                # PSUM [128,TQ] f32 -> fp16 copy -> RoPE rotation on DVE
                # (all-fp16 SBUF operands; inputs pairwise share a base
                # partition as the ALU engines require)
                ts4 = slice(t4 * TQ, (t4 + 1) * TQ)
                qk16 = rope_p.tile([128, TQ], F16, tag="qk16")
                nc.scalar.copy(qk16[:], ps[:])
                dst = qkt_sb[f8][t4]
                t_a = rope_p.tile([64, TQ], F16, tag="ta", bufs=1)
                t_b = rope_p.tile([64, TQ], F16, tag="tb", bufs=1)
                nc.vector.tensor_tensor(t_a[:], qk16[0:64, :],
                                        cs_sb[0:64, ts4], op=AluOpType.mult)
                nc.vector.tensor_tensor(t_b[:], qk16[64:128, :],
                                        cs_sb[64:128, ts4], op=AluOpType.mult)
                nc.vector.tensor_tensor(dst[0:64, :], t_a[:], t_b[:],
                                        op=AluOpType.subtract)
                t_c = rope_p.tile([64, TQ], F16, tag="tc", bufs=1)
                t_d = rope_p.tile([64, TQ], F16, tag="td", bufs=1)
                nc.vector.tensor_tensor(t_c[:], qk16[0:64, :],
                                        csd_sb[0:64, ts4], op=AluOpType.mult)
                nc.vector.tensor_tensor(t_d[:], qk16[64:128, :],
                                        csd_sb[64:128, ts4], op=AluOpType.mult)
                nc.vector.tensor_tensor(dst[64:128, :], t_c[:], t_d[:],
                                        op=AluOpType.add)

            # ---------------- DMA preamble (priority order) ----------------
            # dma_start issue costs ~0.6us on the sync queue (8 outstanding),
            # so keep the count low and issue the first compute's chunks
            # before anything else.
            for e in range(8):
                xt = xt_p.tile([128, TQ], F16, tag=f"xt{e}")
                nc.sync.dma_start(xt[:], xT[e * 128:(e + 1) * 128, 0:TQ])
                xt_sb[(0, e)] = xt
                nc.sync.dma_start(wqk_sb[e][:], wqk[e])
            nc.sync.dma_start(cs_sb[:], csx[:])
            nc.sync.dma_start(csd_sb[:], csx2[:])
            nc.sync.dma_start(mask_sb[:], maskx[:])
            nc.sync.dma_start(ones_sb[:], onesx[:])
            for e in range(8, EC):
                xt = xt_p.tile([128, TQ], F16, tag=f"xt{e}")
                nc.sync.dma_start(xt[:], xT[e * 128:(e + 1) * 128, 0:TQ])
                xt_sb[(0, e)] = xt
                nc.sync.dma_start(wqk_sb[e][:], wqk[e])
            for e in range(EC):
                nc.sync.dma_start(wv_sb[e][:], wv[e])

            # ---------------- slot 0: QKV for t4=0, e-major ----------------
            # Two e-major half-passes (f8 0..3 then 4..7) so the RoPE
            # evictions of each half drain on ACT/DVE under the next pass's
            # matmuls instead of bunching at a single loop end.
            with tc.tile_pool(name="ps0", bufs=1, space="PSUM") as ps0:
                qkps = [ps0.tile([128, TQ], F32, tag=f"qk{f8}",
                                 name=f"qkps{f8}") for f8 in range(2 * HPC)]
                for e in range(EC):
                    for f8 in range(HPC):
                        nc.tensor.matmul(
                            qkps[f8][:],
                            wqk_sb[e][:, f8 * 128:(f8 + 1) * 128],
                            xt_sb[(0, e)][:],
                            start=(e == 0), stop=(e == EC - 1),
                            skip_group_check=True,
                        )
                # prefetch next x slab + wout while evictions/V run
                dma_x_slab(1)
                for h in range(HPC):
                    nc.sync.dma_start(wo_sb[h][:], wout[h])
                for f8 in range(HPC):
                    rope_evict(qkps[f8], f8, 0)
                for e in range(EC):
                    for f8 in range(HPC, 2 * HPC):
                        nc.tensor.matmul(
                            qkps[f8][:],
                            wqk_sb[e][:, f8 * 128:(f8 + 1) * 128],
                            xt_sb[(0, e)][:],
                            start=(e == 0), stop=(e == EC - 1),
                            skip_group_check=True,
                        )
                for f8 in range(HPC, 2 * HPC):
                    rope_evict(qkps[f8], f8, 0)
                for i in range(4):
                    ps = ps0.tile([128, FL], F32, tag=f"qk{i}",
                                  name=f"vps{i}")
                    for e in range(EC):
                        nc.tensor.matmul(
                            ps[:], xt_sb[(0, e)][:, i * 128:(i + 1) * 128],
                            wv_sb[e][:],
                            start=(e == 0), stop=(e == EC - 1),
                            skip_group_check=True,
                        )
                    nc.scalar.copy(v_sb[i][:], ps[:])
                dma_x_slab(2)

            # ---------------- slots 1..4: pipelined SDPA ----------------
            # ps_qkv (2 banks) lives only through slot 2; its banks are then
            # recycled into a 2-deep yps ring for slots 3-4 so a head's first
            # AV matmul never waits the previous head's normalize (DVE).
            with tc.tile_pool(name="ps_sc", bufs=3, space="PSUM") as ps_sc, \
                 tc.tile_pool(name="ps_y", bufs=1, space="PSUM") as ps_y, \
                 tc.tile_pool(name="ps_o", bufs=2, space="PSUM") as ps_o:
                ps_qkv_cm = tc.tile_pool(name="ps_qkv", bufs=2, space="PSUM")
                ps_qkv = ps_qkv_cm.__enter__()
                ps_y_cur = [ps_y]

                def qk_chain(f8, t4):
                    ps = ps_qkv.tile([128, TQ], F32, tag="qkv",
                                     name=f"qkch_{f8}_{t4}")
                    for e in range(EC):
                        nc.tensor.matmul(
                            ps[:], wqk_sb[e][:, f8 * 128:(f8 + 1) * 128],
                            xt_sb[(t4, e)][:],
                            start=(e == 0), stop=(e == EC - 1),
                            skip_group_check=True,
                        )
                    rope_evict(ps, f8, t4)

                def v_chain(i, t4):
                    tk = 4 * t4 + i
                    ps = ps_qkv.tile([128, FL], F32, tag="qkv",
                                     name=f"vch_{tk}")
                    for e in range(EC):
                        nc.tensor.matmul(
                            ps[:], xt_sb[(t4, e)][:, i * 128:(i + 1) * 128],
                            wv_sb[e][:],
                            start=(e == 0), stop=(e == EC - 1),
                            skip_group_check=True,
                        )
                    nc.scalar.copy(v_sb[tk][:], ps[:])

                def proj_unit(tqb, nb, alt=False, split=False):
                    ps = ps_o.tile([128, TQ], F32, tag="o",
                                   name=f"ops_{tqb}_{nb}")
                    for h in range(HPC):
                        nc.tensor.matmul(
                            ps[:],
                            yt_sb[h][tqb // 4][:, (tqb % 4) * 128:
                                               (tqb % 4 + 1) * 128],
                            wo_sb[h][:, nb * TQ:(nb + 1) * TQ],
                            start=(h == 0), stop=(h == HPC - 1),
                            skip_group_check=True,
                        )
                    osb = o_ev.tile([128, TQ], F16, tag="osb",
                                    name=f"osb_{tqb}_{nb}")
                    if split:
                        # halves on both engines in parallel for the final
                        # flush latency
                        nc.scalar.copy(osb[:, 0:256], ps[:, 0:256])
                        nc.vector.tensor_copy(osb[:, 256:TQ], ps[:, 256:TQ])
                        for p in range(2):
                            c0 = nb * TQ + p * 256
                            nc.sync.dma_start(
                                out[tqb * 128:(tqb + 1) * 128, c0:c0 + 256],
                                osb[:, p * 256:(p + 1) * 256])
                        return
                    if alt and nb % 2 == 1:
                        nc.vector.tensor_copy(osb[:], ps[:])
                    else:
                        nc.scalar.copy(osb[:], ps[:])
                    nc.sync.dma_start(
                        out[tqb * 128:(tqb + 1) * 128,
                            nb * TQ:(nb + 1) * TQ], osb[:])

                def proj_chunk(tqb, alt=False):
                    # out rows tqb*128..+128, partial over local heads
                    for nb in range(NTQ):
                        proj_unit(tqb, nb, alt)

                def pop(fillers, n=1):
                    for _ in range(n):
                        if fillers:
                            fillers.pop(0)()

                def sdpa_head(h, tq, fillers, drain=False, nfill=1):
                    nblk = 4 * tq + 4
                    ngrp = tq + 1
                    qs = qkt_sb[h][tq]
                    yps = ps_y_cur[0].tile([128, TQ], F32, tag="y",
                                           name=f"yps_{h}_{tq}")
                    es_of = {}
                    acc = None

                    def scores_grp(g):
                        for r4 in range(4):
                            tk = 4 * g + r4
                            dr = tk - 4 * tq
                            lo = 128 * dr if dr > 0 else 0
                            sps = ps_sc.tile([128, TQ], F32, tag="sc",
                                             name=f"sps_{h}_{tq}_{tk}")
                            nc.tensor.matmul(
                                sps[:, lo:TQ],
                                qkt_sb[HPC + h][tk // 4][:, (tk % 4) * 128:
                                                         (tk % 4 + 1) * 128],
                                qs[:, lo:TQ], start=True, stop=True,
                                skip_group_check=True,
                            )
                            es = es_p.tile([128, TQ], F16, tag=f"es{r4}",
                                           name=f"es_{h}_{tq}_{tk}")
                            nc.scalar.activation(es[:, lo:TQ], sps[:, lo:TQ],
                                                 EXP, scale=SCALE)
                            if dr >= 0:
                                nc.vector.tensor_tensor(
                                    es[:, lo:lo + 128], es[:, lo:lo + 128],
                                    mask_sb[:], op=AluOpType.mult)
                            es_of[tk] = es

                    def av_grp(g):
                        for r4 in range(4):
                            tk = 4 * g + r4
                            dr = tk - 4 * tq
                            lo = 128 * dr if dr > 0 else 0
                            nc.tensor.matmul(
                                yps[:, lo:TQ],
                                v_sb[tk][:, h * 128:(h + 1) * 128],
                                es_of[tk][:, lo:TQ],
                                start=(tk == 0), stop=(tk == nblk - 1),
                                skip_group_check=True,
                            )

                    def dsum_grp(g):
                        # fp16 sum tree of the 4 (full) blocks of group g
                        nonlocal acc
                        e0, e1, e2, e3 = (es_of[4 * g + r] for r in range(4))
                        pra = dn_p.tile([128, TQ], F16, tag="pra")
                        prb = dn_p.tile([128, TQ], F16, tag="prb")
                        nc.vector.tensor_tensor(pra[:], e0[:], e1[:],
                                                op=AluOpType.add)
                        nc.vector.tensor_tensor(prb[:], e2[:], e3[:],
                                                op=AluOpType.add)
                        if acc is None:
                            acc = dn_p.tile([128, TQ], F16, tag="acc")
                            nc.vector.tensor_tensor(acc[:], pra[:], prb[:],
                                                    op=AluOpType.add)
                        else:
                            gs = dn_p.tile([128, TQ], F16, tag="gs", bufs=1)
                            nc.vector.tensor_tensor(gs[:], pra[:], prb[:],
                                                    op=AluOpType.add)
                            nc.vector.tensor_tensor(acc[:], acc[:], gs[:],
                                                    op=AluOpType.add)

                    def dsum_diag():
                        # ragged sum of the diagonal band into a fresh tile
                        # (read-only on es tiles, so the all_reduce below can
                        # overlap the diagonal AV matmuls)
                        nonlocal acc
                        d0, d1, d2, d3 = (es_of[4 * tq + r] for r in range(4))
                        prd = dn_p.tile([128, TQ], F16, tag="prd", bufs=1)
                        nc.vector.tensor_copy(prd[:, 0:128], d0[:, 0:128])
                        nc.vector.tensor_tensor(prd[:, 128:TQ],
                                                d0[:, 128:TQ], d1[:, 128:TQ],
                                                op=AluOpType.add)
                        nc.vector.tensor_tensor(prd[:, 256:TQ],
                                                prd[:, 256:TQ], d2[:, 256:TQ],
                                                op=AluOpType.add)
                        nc.vector.tensor_tensor(prd[:, 384:TQ],
                                                prd[:, 384:TQ], d3[:, 384:TQ],
                                                op=AluOpType.add)
                        if acc is None:
                            return prd
                        nc.vector.tensor_tensor(acc[:], acc[:], prd[:],
                                                op=AluOpType.add)
                        return acc

                    for g in range(ngrp):
                        scores_grp(g)
                        if g > 0:
                            av_grp(g - 1)
                            dsum_grp(g - 1)
                        pop(fillers, nfill)
                    dacc = dsum_diag()
                    rcp = dn_p.tile([128, TQ], F32, tag="rcp",
                                    name=f"rcp_{h}_{tq}")
                    if (h, tq) == (HPC - 1, NTQ - 1):
                        # final head gates the tail: use the short
                        # ones-matmul + broadcast chain instead of the
                        # ~3.5us gpsimd all_reduce
                        dps = ps_y_cur[0].tile([1, TQ], F32, tag="y",
                                               name="dps_tail")
                        nc.tensor.matmul(dps[:], ones_sb[:], dacc[:],
                                         start=True, stop=True,
                                         skip_group_check=True)
                        rrow = dn_p.tile([1, TQ], F32, tag="rrow", bufs=1)
                        nc.vector.reciprocal_approx_fast(rrow[:], dps[:])
                        nc.gpsimd.partition_broadcast(rcp[:], rrow[:])
                    else:
                        rb = dn_p.tile([128, TQ], F32, tag="rb",
                                       name=f"rb_{h}_{tq}")
                        nc.gpsimd.partition_all_reduce(
                            rb[:], dacc[:], channels=128,
                            reduce_op=bass_isa.ReduceOp.add)
                        nc.vector.reciprocal_approx_fast(rcp[:], rb[:])
                    av_grp(ngrp - 1)
                    pop(fillers, nfill)
                    nc.vector.tensor_tensor(yt_sb[h][tq][:], yps[:], rcp[:],
                                            op=AluOpType.mult)
                    if drain:
                        while fillers:
                            pop(fillers)

                # QKV filler order: KT chains (f8 4..7) interleaved with QT
                # (f8 0..3) so the next slot's first score matmuls find both
                # its KT and the early heads' QT already RoPE'd; V last (its
                # consumers are each head's final AV group next slot).
                def qkv_fillers(t4):
                    fs = []
                    for k in range(4):
                        fs.append(lambda f8=4 + k, t=t4: qk_chain(f8, t))
                        fs.append(lambda f8=k, t=t4: qk_chain(f8, t))
                    fs += [lambda i=i, t=t4: v_chain(i, t) for i in range(4)]
                    return fs

                # slot 1: sdpa tq=0, filler = QKV t4=1
                fillers = qkv_fillers(1)
                for h in range(HPC):
                    sdpa_head(h, 0, fillers, drain=(h == HPC - 1), nfill=2)
                dma_x_slab(3)

                # slot 2: sdpa tq=1, filler = QKV t4=2 AND t4=3 (pulled
                # forward so the exp-heavy tq=3 heads can spread over two
                # slots below)
                fillers = qkv_fillers(2) + qkv_fillers(3)
                for h in range(HPC):
                    sdpa_head(h, 1, fillers, drain=(h == HPC - 1), nfill=2)

                ps_qkv_cm.__exit__(None, None, None)
                ps_y34_cm = tc.tile_pool(name="ps_y34", bufs=2, space="PSUM")
                ps_y_cur[0] = ps_y34_cm.__enter__()

                # slot 3: sdpa tq=2 (4 heads) + sdpa tq=3 (heads 0,1),
                # filler = proj 0..7 rationed per head
                proj_fill = [lambda t=t: proj_chunk(t) for t in range(0, 8)]
                for h in range(HPC):
                    sdpa_head(h, 2, [proj_fill.pop(0)])
                sdpa_head(0, 3, [proj_fill.pop(0) for _ in range(2)])
                sdpa_head(1, 3, [proj_fill.pop(0) for _ in range(2)])

                # slot 4: sdpa tq=3 heads 2,3 with proj 8..11 rationed;
                # tail = proj 12..15
                sdpa_head(2, 3, [lambda t=t: proj_chunk(t)
                                 for t in range(8, 10)])
                # final head: one proj unit per group boundary keeps the exp
                # chase covered at fine grain, and a full chunk lands right
                # after its last AV to cover the denominator chain
                h3_fill = [lambda nb=nb: proj_unit(10, nb) for nb in range(4)]
                h3_fill.append(lambda: proj_chunk(11))
                sdpa_head(3, 3, h3_fill)
                for t in range(12, 16):
                    proj_chunk(t, alt=True)
                ps_y34_cm.__exit__(None, None, None)

    nc.compile()
    return nc


def _host_tables():
    positions = np.arange(T, dtype=np.float64)
    inv_freq = 1.0 / (THETA ** (np.arange(0, D, 2, dtype=np.float64) / D))
    freqs = np.outer(positions, inv_freq)          # [T, 64]
    cs = np.concatenate([np.cos(freqs).T, np.sin(freqs).T]).astype(np.float16)
    cs2 = np.concatenate([np.sin(freqs).T, np.cos(freqs).T]).astype(np.float16)
    p = np.arange(128)[:, None]
    j = np.arange(128)[None, :]
    mask = (p <= j).astype(np.float16)             # [128, 128] triangle
    return cs, cs2, mask


def kernel(x, W_qkv, W_out):
    global _compiled
    if _compiled is None:
        _compiled = _build()
    nc = _compiled

    x = np.ascontiguousarray(np.asarray(x, dtype=np.float32))
    W_qkv = np.asarray(W_qkv, dtype=np.float32)
    W_out = np.asarray(W_out, dtype=np.float32)

    cs, cs2, mask = _host_tables()

    perm = np.concatenate([np.arange(0, D, 2), np.arange(1, D, 2)])  # de-interleave

    in_maps = []
    for c in range(N_CORES):
        b, tp = divmod(c, TP)
        heads = np.arange(tp * HPC, (tp + 1) * HPC)
        qk_cols = np.concatenate(
            [h * D + perm for h in heads] + [E + h * D + perm for h in heads]
        )
        v_cols = np.concatenate([2 * E + h * D + np.arange(D) for h in heads])
        wqk_l = np.ascontiguousarray(W_qkv[:, qk_cols]).reshape(EC, 128, 2 * FL)
        wv_l = np.ascontiguousarray(W_qkv[:, v_cols]).reshape(EC, 128, FL)
        wout_l = np.ascontiguousarray(
            W_out.reshape(N_HEAD, D, E)[heads].reshape(HPC, 128, E)
        )
        in_maps.append({
            "xT": np.ascontiguousarray(x[b].T).astype(np.float16),
            "wqk": wqk_l.astype(np.float16),
            "wv": wv_l.astype(np.float16),
            "wout": wout_l.astype(np.float16),
            "csx": cs,
            "csx2": cs2,
            "maskx": mask,
            "onesx": np.ones((128, 1), np.float16),
        })

    global _last_in_maps
    _last_in_maps = in_maps
    res = bass_utils.run_bass_kernel_spmd(nc, in_maps, core_ids=list(range(N_CORES)))
    out = np.zeros((B, T, E), dtype=np.float32)
    for c in range(N_CORES):
        out[c // TP] += res.results[c]["out"].astype(np.float32)
    return out
